# revision 9
# baseline (speedup 1.0000x reference)
"""Multi-head cross-attention TRN2 Bass kernel, sharded over 8 NeuronCores.

Problem (nn_MultiHeadCrossAttention): B=2, Sq=1024, Skv=4096 (text+image+
audio+video), hidden=1024, heads=16, head_dim=64, out=4096.

Sharding: core c = 4*b + hg handles batch b and head-group hg (4 heads).
Per core (all matmuls in float32r: ~bf16 speed, ~1e-4 accuracy):
  QT proj:  QT[d,sq]   = Wq_g  @ ff[b].T      (ffT streamed, contraction 4096)
  KT proj:  KT[d,kv]   = Wk_g  @ kv[b].T      (kvT streamed, contraction 1024)
  V  proj:  V[kv,d]    = kv[b] @ Wv_g.T       (natural layout, 65th col = ones)
  scores^T: S[kv,sq]   = K^T q  (row-tiled K=64 matmul pairs)
  softmax:  P = exp(S/8) (no max-subtract: |scores| <~ 3 for this data)
  PV:       att[d,sq] += V_ext^T @ P  (M=65: row 64 accumulates denominator)
  norm:     att = att * recip(den) (K=1 broadcast matmul expands recip row)
  out-proj: outT[j,sq] = Wo[:, fslice].T.T @ attT  -> partial over f-slice
Host sums the 4 per-batch partials and adds bo.
"""

import numpy as np

import bass_rust
import concourse.bass as bass
import concourse.mybir as mybir
import concourse.tile as tile
from concourse.bass_utils import run_bass_kernel_spmd
from concourse.vector_clock import ScopedClock

# ---------------------------------------------------------------------------
# Workarounds for walrus per-instruction sync-wait caps (this walrus build
# rejects instructions carrying more waits than the ISA slot count; Tile's
# sem assignment can attach more). Split excess waits onto single-wait nops.
# ---------------------------------------------------------------------------
import re as _re

_VC_RE = _re.compile(r"VectorClock\(\[([0-9, ]*)\]\)")


def _vc_values(vc):
    m = _VC_RE.match(repr(vc))
    assert m, repr(vc)
    s = m.group(1).strip()
    return [int(x) for x in s.split(",")] if s else []


def _split_excess_waits(tc, ordered_instructions_by_block, max_waits=1):
    nc = tc.nc
    for _bb, insts in ordered_instructions_by_block.items():
        out = []
        for inst in insts:
            si = inst.sync_info
            waits = list(si.on_wait) if si and si.on_wait else []
            if len(waits) > max_waits:
                keep = waits[:max_waits]
                for w in waits[max_waits:]:
                    nop = mybir.InstNoOp(
                        name=nc.get_next_instruction_name(), ins=[], outs=[]
                    )
                    nop.engine = inst.engine
                    nop.sync_info = bass_rust.SyncInfo(on_wait=[w], on_update=[])
                    nc.register_instruction(nop)
                    out.append(nop)
                inst.sync_info = bass_rust.SyncInfo(
                    on_wait=keep, on_update=list(si.on_update or [])
                )
            out.append(inst)
        insts[:] = out


_orig_lower = tile.TileContext._lower_ordered_insts


def _lower_with_split(self, postordered_blocks):
    _split_excess_waits(self, postordered_blocks)
    return _orig_lower(self, postordered_blocks)


def _drain_and_barrier_split(self, tick_clock, wait_clock):
    vals = _vc_values(tick_clock.global_clock)
    for proc_idx, tick in enumerate(vals):
        if tick <= 0:
            continue
        single = [0] * len(vals)
        single[proc_idx] = tick
        nop_inst = self.nc.sync.nop(nofuse=True, hint=f"drain_wait_p{proc_idx}")
        wait_clock.add_sem_waits(
            nop_inst.ins, ScopedClock({None: bass_rust.VectorClock(single)})
        )
    self.nc.sync.drain()
    self.nc.all_engine_barrier()
    assert self.sems is not None
    popped = self.nc._tile_sem_poison_stack.pop()
    assert popped is self._sem_poison
    self.nc.clear_and_free_semaphores(list(self.sems.allocated().values()))
    self.nc.all_engine_barrier()


tile.TileContext._lower_ordered_insts = _lower_with_split
tile.TileContext._drain_and_barrier = _drain_and_barrier_split

# ---------------------------------------------------------------------------
# Problem constants (hardcoded per contract)
# ---------------------------------------------------------------------------
B = 2
SQ = 1024
SKV = 4096
HID = 1024
HEADS = 16
DH = 64
DOUT = 4096
NCORES = 8
HG = 4  # head-groups (cores per batch)
GHEADS = HEADS // HG  # heads per group = 4
GF = GHEADS * DH  # feature slice per group = 256
NPAIR = GHEADS // 2  # head pairs per group = 2

F32 = mybir.dt.float32
F32R = mybir.dt.float32r
BF16 = mybir.dt.bfloat16
FP16 = mybir.dt.float16
DT_MM = BF16  # matmul operand dtype: BF16 (fast ldweights) or F32R (accuracy)
NP_MM = "bfloat16"  # host-side dtype name for DT_MM inputs
Exp = mybir.ActivationFunctionType.Exp
MUL = mybir.AluOpType.mult
ADD = mybir.AluOpType.add

NKVT = SKV // 128  # 32 kv tiles
NKVB = 8  # kv blocks (512 wide)
NFT_Q = 4096 // 128  # 32 contraction tiles for Q proj
NFT_KV = HID // 128  # 8 contraction tiles for K/V proj
NSQH = SQ // 512  # 2 sq halves
NJT = DOUT // 128  # 32 output row tiles

_NC_CACHE = {}


def build():
    if "nc" in _NC_CACHE:
        return _NC_CACHE["nc"]
    nc = bass.Bass()

    fft = nc.declare_dram_parameter("fft", [4096, SQ], DT_MM, isOutput=False)
    kvt = nc.declare_dram_parameter("kvt", [HID, SKV], DT_MM, isOutput=False)
    wqt = nc.declare_dram_parameter("wqt", [4096, GF], DT_MM, isOutput=False)
    wkt = nc.declare_dram_parameter("wkt", [HID, GF], DT_MM, isOutput=False)
    wvt = nc.declare_dram_parameter("wvt", [HID, GF], DT_MM, isOutput=False)
    wot = nc.declare_dram_parameter("wot", [GF, DOUT], DT_MM, isOutput=False)
    bq = nc.declare_dram_parameter("bq", [128, NPAIR], F32, isOutput=False)
    bk = nc.declare_dram_parameter("bk", [128, NPAIR], F32, isOutput=False)
    bv = nc.declare_dram_parameter("bv", [128, NPAIR], F32, isOutput=False)
    outp = nc.declare_dram_parameter("outp", [DOUT, SQ], FP16, isOutput=True)

    with tile.TileContext(nc) as tc:
        with (
            tc.tile_pool(name="hold", bufs=1) as hold,
            tc.tile_pool(name="misc", bufs=1) as misc,
        ):
            # ---- long-lived tiles ----
            wkt_r = hold.tile([128, NFT_KV, NPAIR, 128], DT_MM, tag="wkt")
            nc.sync.dma_start(
                out=wkt_r[:],
                in_=wkt.rearrange("(ft p) (pr d) -> p ft pr d", p=128, pr=NPAIR),
            )
            wvt_r = hold.tile([128, NFT_KV, GF], DT_MM, tag="wvt")
            nc.sync.dma_start(
                out=wvt_r[:], in_=wvt.rearrange("(ft p) d -> p ft d", p=128)
            )
            wot_r = hold.tile([128, NPAIR, DOUT], DT_MM, tag="wot")
            nc.sync.dma_start(
                out=wot_r[:], in_=wot.rearrange("(pr p) j -> p pr j", p=128)
            )
            bq_t = misc.tile([128, NPAIR], F32, tag="bq")
            nc.sync.dma_start(out=bq_t[:], in_=bq[:])
            bk_t = misc.tile([128, NPAIR], F32, tag="bk")
            nc.sync.dma_start(out=bk_t[:], in_=bk[:])
            bv_t = misc.tile([128, NPAIR], F32, tag="bv")
            nc.sync.dma_start(out=bv_t[:], in_=bv[:])

            ones_f = misc.tile([128, GHEADS], F32, tag="ones_f")
            nc.vector.memset(ones_f[:], 1.0)
            ones_row = misc.tile([1, DH], DT_MM, tag="ones_row")
            nc.vector.tensor_copy(ones_row[:], ones_f[0:1, 0:1].broadcast_to([1, DH]))

            qt_r = hold.tile([128, NPAIR, SQ], DT_MM, tag="qt")
            kt_r = hold.tile([128, NPAIR, SKV], DT_MM, tag="kt")
            v_r = hold.tile([128, NKVT, GHEADS, DH + 1], DT_MM, tag="v")
            att_r = hold.tile([128, NPAIR, SQ], DT_MM, tag="att")

            # ================= Phase A: QT projection =================
            with (
                nc.named_scope("phaseA_qt"),
                tc.tile_pool(name="ffts", bufs=6) as ffts,
                tc.tile_pool(name="wqs", bufs=4) as wqs,
                tc.tile_pool(name="psA", bufs=4, space="PSUM") as psA,
            ):
                qt_ps = [
                    psA.tile([128, 512], F32, tag="psA", name=f"qt_ps{i}")
                    for i in range(4)
                ]  # (pair, sqh)
                for kt in range(NFT_Q):
                    fft_t = ffts.tile([128, SQ], DT_MM, tag="fft")
                    nc.sync.dma_start(
                        out=fft_t[:], in_=fft[128 * kt : 128 * (kt + 1), :]
                    )
                    wq_t = wqs.tile([128, NPAIR, 128], DT_MM, tag="wq")
                    nc.sync.dma_start(
                        out=wq_t[:],
                        in_=wqt[128 * kt : 128 * (kt + 1), :].rearrange(
                            "p (pr d) -> p pr d", pr=NPAIR
                        ),
                    )
                    for pr in range(NPAIR):
                        for sh in range(NSQH):
                            nc.tensor.matmul(
                                qt_ps[pr * NSQH + sh][:],
                                wq_t[:, pr, :],
                                fft_t[:, 512 * sh : 512 * (sh + 1)],
                                start=(kt == 0),
                                stop=(kt == NFT_Q - 1),
                            )
                for pr in range(NPAIR):
                    for sh in range(NSQH):
                        nc.vector.tensor_scalar(
                            qt_r[:, pr, 512 * sh : 512 * (sh + 1)],
                            qt_ps[pr * NSQH + sh][:],
                            bq_t[:, pr : pr + 1],
                            None,
                            ADD,
                        )

            # ============ Phase B: KT + V projections (kv blocks) ============
            with (
                nc.named_scope("phaseB_kv"),
                tc.tile_pool(name="kvs", bufs=2) as kvs,
                tc.tile_pool(name="psB", bufs=4, space="PSUM") as psB,
            ):
                for kb in range(NKVB):
                    kv_t = kvs.tile([128, NFT_KV, 512], DT_MM, tag="kv")
                    nc.sync.dma_start(
                        out=kv_t[:],
                        in_=kvt[:, 512 * kb : 512 * (kb + 1)].rearrange(
                            "(ft p) n -> p ft n", p=128
                        ),
                    )
                    for pr in range(NPAIR):
                        kt_ps = psB.tile([128, 512], F32, tag="psB")
                        for ft in range(NFT_KV):
                            nc.tensor.matmul(
                                kt_ps[:],
                                wkt_r[:, ft, pr, :],
                                kv_t[:, ft, :],
                                start=(ft == 0),
                                stop=(ft == NFT_KV - 1),
                            )
                        nc.vector.tensor_scalar(
                            kt_r[:, pr, 512 * kb : 512 * (kb + 1)],
                            kt_ps[:],
                            bk_t[:, pr : pr + 1],
                            None,
                            ADD,
                        )
                    for kl in range(4):
                        kvt_i = kb * 4 + kl
                        v_ps = psB.tile([128, GF], F32, tag="psB")
                        for ft in range(NFT_KV):
                            nc.tensor.matmul(
                                v_ps[:],
                                kv_t[:, ft, 128 * kl : 128 * (kl + 1)],
                                wvt_r[:, ft, :],
                                start=(ft == 0),
                                stop=(ft == NFT_KV - 1),
                            )
                        nc.vector.tensor_copy(
                            v_r[:, kvt_i, :, 0:DH],
                            v_ps.rearrange("p (h d) -> p h d", h=GHEADS),
                        )
                        nc.vector.tensor_copy(
                            v_r[:, kvt_i, :, DH : DH + 1], ones_f[:, :]
                        )

            # ================= Phase C: attention =================
            # Per pair: kv-loop with both sq halves interleaved. One
            # LDWEIGHTS serves both halves' matmuls; scores for each head
            # land in a [128, 2x512] psum tile (2 banks) so each exp
            # covers 1024 elements/lane (amortizes ACT's +352cyc/inst).
            # PV runs one kv-tile behind scores (software pipeline) so
            # the exp latency never stalls the PE.
            with (
                nc.named_scope("phaseC_attn"),
                tc.tile_pool(name="pp", bufs=2) as pp,
                tc.tile_pool(name="nrm", bufs=2) as nrm,
                tc.tile_pool(name="psS", bufs=1, space="PSUM") as psS,
                tc.tile_pool(name="psAtt", bufs=1, space="PSUM") as psAtt,
            ):
                for pr in range(NPAIR):
                    att_ps = [
                        psAtt.tile(
                            [DH + 1, 512], F32, tag=f"att{h}{sh}",
                            name=f"att{pr}_{h}_{sh}",
                        )
                        for h in range(2)
                        for sh in range(NSQH)
                    ]  # index h * NSQH + sh

                    def pv(kv, pq, pr=pr, att_ps=att_ps):
                        q0, q1 = pq
                        for h, q in ((0, q0), (1, q1)):
                            for sh in range(NSQH):
                                nc.tensor.matmul(
                                    att_ps[h * NSQH + sh][:],
                                    v_r[:, kv, 2 * pr + h, :],
                                    q[:, sh, :],
                                    start=(kv == 0),
                                    stop=(kv == NKVT - 1),
                                )

                    pq = []  # pending (kv, p0, p1) awaiting PV
                    for kv in range(NKVT):
                        s0 = psS.tile([128, NSQH, 512], F32, tag="sh0")
                        s1 = psS.tile([128, NSQH, 512], F32, tag="sh1")
                        kv_sl = slice(128 * kv, 128 * (kv + 1))
                        for sh in range(NSQH):
                            nc.tensor.matmul(
                                s0[:, sh, :],
                                kt_r[0:DH, pr, kv_sl],
                                qt_r[0:DH, pr, 512 * sh : 512 * (sh + 1)],
                                start=True,
                                stop=True,
                            )
                        for sh in range(NSQH):
                            nc.tensor.matmul(
                                s1[:, sh, :],
                                kt_r[DH:128, pr, kv_sl],
                                qt_r[DH:128, pr, 512 * sh : 512 * (sh + 1)],
                                start=True,
                                stop=True,
                            )
                        p0 = pp.tile([128, NSQH, 512], DT_MM, tag="p0")
                        p1 = pp.tile([128, NSQH, 512], DT_MM, tag="p1")
                        nc.scalar.activation(p0[:], s0[:], Exp, scale=0.125)
                        nc.scalar.activation(p1[:], s1[:], Exp, scale=0.125)
                        pq.append((kv, (p0, p1)))
                        if kv >= 1:
                            pkv, pqt = pq.pop(0)
                            pv(pkv, pqt)
                    pkv, pqt = pq.pop(0)
                    pv(pkv, pqt)

                    # normalize: per (head, sh)
                    for h in range(2):
                        for sh in range(NSQH):
                            att_h = att_ps[h * NSQH + sh]
                            sq_sl = slice(512 * sh, 512 * (sh + 1))
                            rec = nrm.tile(
                                [1, 512], DT_MM, tag="rec", name=f"rec{pr}{h}{sh}"
                            )
                            with nc.allow_low_precision(reason="softmax recip"):
                                nc.vector.reciprocal(rec[:], att_h[DH : DH + 1, :])
                            rb = psS.tile(
                                [DH, 512], F32, tag="sh0", name=f"rb{pr}{h}{sh}"
                            )
                            nc.tensor.matmul(
                                rb[:], ones_row[0:1, :], rec[0:1, :],
                                start=True, stop=True,
                            )
                            rb_sb = nrm.tile(
                                [DH, 512], F32, tag="rbsb", name=f"rbsb{pr}{h}{sh}"
                            )
                            nc.vector.tensor_copy(rb_sb[:], rb[:])
                            mulx = nrm.tile(
                                [DH, 512], F32, tag="mulx", name=f"mulx{pr}{h}{sh}"
                            )
                            nc.vector.tensor_tensor(
                                mulx[:], att_h[0:DH, :], rb_sb[:], MUL
                            )
                            nc.vector.tensor_scalar(
                                att_r[64 * h : 64 * (h + 1), pr, sq_sl],
                                mulx[:],
                                bv_t[64 * h : 64 * (h + 1), pr : pr + 1],
                                None,
                                ADD,
                            )

            # ================= Phase D: out projection =================
            with (
                nc.named_scope("phaseD_out"),
                tc.tile_pool(name="osb", bufs=3) as osb,
                tc.tile_pool(name="psD", bufs=4, space="PSUM") as psD,
            ):
                for jt in range(NJT):
                    o_ps = [psD.tile([128, 512], F32, tag="psD", name=f"o_ps{jt}_{i}") for i in range(NSQH)]
                    j_sl = slice(128 * jt, 128 * (jt + 1))
                    for pr in range(NPAIR):
                        for sh in range(NSQH):
                            nc.tensor.matmul(
                                o_ps[sh][:],
                                wot_r[:, pr, j_sl],
                                att_r[:, pr, 512 * sh : 512 * (sh + 1)],
                                start=(pr == 0),
                                stop=(pr == NPAIR - 1),
                            )
                    o_sb = osb.tile([128, SQ], FP16, tag="osb")
                    for sh in range(NSQH):
                        nc.vector.tensor_copy(
                            o_sb[:, 512 * sh : 512 * (sh + 1)], o_ps[sh][:]
                        )
                    nc.sync.dma_start(out=outp[j_sl, :], in_=o_sb[:])

    _NC_CACHE["nc"] = nc
    return nc


def _make_in_maps(inputs):
    ff = np.asarray(inputs["fused_features"], dtype=np.float32)
    kv_in = np.concatenate(
        [
            np.asarray(inputs["text"], dtype=np.float32),
            np.asarray(inputs["image"], dtype=np.float32),
            np.asarray(inputs["audio"], dtype=np.float32),
            np.asarray(inputs["video"], dtype=np.float32),
        ],
        axis=1,
    )
    Wq = np.asarray(inputs["Wq"], dtype=np.float32)
    Wk = np.asarray(inputs["Wk"], dtype=np.float32)
    Wv = np.asarray(inputs["Wv"], dtype=np.float32)
    Wo = np.asarray(inputs["Wo"], dtype=np.float32)
    bq = np.asarray(inputs["bq"], dtype=np.float32)
    bk = np.asarray(inputs["bk"], dtype=np.float32)
    bv = np.asarray(inputs["bv"], dtype=np.float32)

    import ml_dtypes

    np_mm = np.dtype(ml_dtypes.bfloat16) if NP_MM == "bfloat16" else np.float32
    ffT = [np.ascontiguousarray(ff[b].T.astype(np_mm)) for b in range(B)]
    kvT = [np.ascontiguousarray(kv_in[b].T.astype(np_mm)) for b in range(B)]
    WqT = np.ascontiguousarray(Wq.T.astype(np_mm))  # [4096, 1024]
    WkT = np.ascontiguousarray(Wk.T.astype(np_mm))  # [1024, 1024]
    WvT = np.ascontiguousarray(Wv.T.astype(np_mm))
    WoT = np.ascontiguousarray(Wo.T.astype(np_mm))  # [1024, 4096]

    in_maps = []
    for c in range(NCORES):
        b, hg = divmod(c, HG)
        fs = slice(GF * hg, GF * (hg + 1))
        in_maps.append(
            {
                "fft": ffT[b],
                "kvt": kvT[b],
                "wqt": np.ascontiguousarray(WqT[:, fs]),
                "wkt": np.ascontiguousarray(WkT[:, fs]),
                "wvt": np.ascontiguousarray(WvT[:, fs]),
                "wot": np.ascontiguousarray(WoT[fs, :]),
                "bq": np.ascontiguousarray(bq[fs].reshape(NPAIR, 128).T),
                "bk": np.ascontiguousarray(bk[fs].reshape(NPAIR, 128).T),
                "bv": np.ascontiguousarray(bv[fs].reshape(NPAIR, 128).T),
            }
        )
    return in_maps


def _assemble(results, bo):
    out = np.zeros((B, SQ, DOUT), dtype=np.float32)
    for c in range(NCORES):
        b = c // HG
        out[b] += results[c]["outp"].T.astype(np.float32)
    out += np.asarray(bo, dtype=np.float32)
    return out


def run_spmd(inputs, trace=False):
    nc = build()
    in_maps = _make_in_maps(inputs)
    r = run_bass_kernel_spmd(nc, in_maps, list(range(NCORES)), trace=trace)
    return _assemble(r.results, inputs["bo"]), r


def kernel(**inputs) -> np.ndarray:
    out, _ = run_spmd(inputs, trace=False)
    return out


# revision 10
# speedup vs baseline: 1.0004x; 1.0004x over previous
"""Multi-head cross-attention TRN2 Bass kernel, sharded over 8 NeuronCores.

Problem (nn_MultiHeadCrossAttention): B=2, Sq=1024, Skv=4096 (text+image+
audio+video), hidden=1024, heads=16, head_dim=64, out=4096.

Sharding: core c = 4*b + hg handles batch b and head-group hg (4 heads).
Per core (all matmuls in float32r: ~bf16 speed, ~1e-4 accuracy):
  QT proj:  QT[d,sq]   = Wq_g  @ ff[b].T      (ffT streamed, contraction 4096)
  KT proj:  KT[d,kv]   = Wk_g  @ kv[b].T      (kvT streamed, contraction 1024)
  V  proj:  V[kv,d]    = kv[b] @ Wv_g.T       (natural layout, 65th col = ones)
  scores^T: S[kv,sq]   = K^T q  (row-tiled K=64 matmul pairs)
  softmax:  P = exp(S/8) (no max-subtract: |scores| <~ 3 for this data)
  PV:       att[d,sq] += V_ext^T @ P  (M=65: row 64 accumulates denominator)
  norm:     att = att * recip(den) (K=1 broadcast matmul expands recip row)
  out-proj: outT[j,sq] = Wo[:, fslice].T.T @ attT  -> partial over f-slice
Host sums the 4 per-batch partials and adds bo.
"""

import numpy as np

import bass_rust
import concourse.bass as bass
import concourse.mybir as mybir
import concourse.tile as tile
from concourse.bass_utils import run_bass_kernel_spmd
from concourse.vector_clock import ScopedClock

# ---------------------------------------------------------------------------
# Workarounds for walrus per-instruction sync-wait caps (this walrus build
# rejects instructions carrying more waits than the ISA slot count; Tile's
# sem assignment can attach more). Split excess waits onto single-wait nops.
# ---------------------------------------------------------------------------
import re as _re

_VC_RE = _re.compile(r"VectorClock\(\[([0-9, ]*)\]\)")


def _vc_values(vc):
    m = _VC_RE.match(repr(vc))
    assert m, repr(vc)
    s = m.group(1).strip()
    return [int(x) for x in s.split(",")] if s else []


def _split_excess_waits(tc, ordered_instructions_by_block, max_waits=1):
    nc = tc.nc
    for _bb, insts in ordered_instructions_by_block.items():
        out = []
        for inst in insts:
            si = inst.sync_info
            waits = list(si.on_wait) if si and si.on_wait else []
            if len(waits) > max_waits:
                keep = waits[:max_waits]
                for w in waits[max_waits:]:
                    nop = mybir.InstNoOp(
                        name=nc.get_next_instruction_name(), ins=[], outs=[]
                    )
                    nop.engine = inst.engine
                    nop.sync_info = bass_rust.SyncInfo(on_wait=[w], on_update=[])
                    nc.register_instruction(nop)
                    out.append(nop)
                inst.sync_info = bass_rust.SyncInfo(
                    on_wait=keep, on_update=list(si.on_update or [])
                )
            out.append(inst)
        insts[:] = out


_orig_lower = tile.TileContext._lower_ordered_insts


def _lower_with_split(self, postordered_blocks):
    _split_excess_waits(self, postordered_blocks)
    return _orig_lower(self, postordered_blocks)


def _drain_and_barrier_split(self, tick_clock, wait_clock):
    vals = _vc_values(tick_clock.global_clock)
    for proc_idx, tick in enumerate(vals):
        if tick <= 0:
            continue
        single = [0] * len(vals)
        single[proc_idx] = tick
        nop_inst = self.nc.sync.nop(nofuse=True, hint=f"drain_wait_p{proc_idx}")
        wait_clock.add_sem_waits(
            nop_inst.ins, ScopedClock({None: bass_rust.VectorClock(single)})
        )
    self.nc.sync.drain()
    self.nc.all_engine_barrier()
    assert self.sems is not None
    popped = self.nc._tile_sem_poison_stack.pop()
    assert popped is self._sem_poison
    self.nc.clear_and_free_semaphores(list(self.sems.allocated().values()))
    self.nc.all_engine_barrier()


tile.TileContext._lower_ordered_insts = _lower_with_split
tile.TileContext._drain_and_barrier = _drain_and_barrier_split

# ---------------------------------------------------------------------------
# Problem constants (hardcoded per contract)
# ---------------------------------------------------------------------------
B = 2
SQ = 1024
SKV = 4096
HID = 1024
HEADS = 16
DH = 64
DOUT = 4096
NCORES = 8
HG = 4  # head-groups (cores per batch)
GHEADS = HEADS // HG  # heads per group = 4
GF = GHEADS * DH  # feature slice per group = 256
NPAIR = GHEADS // 2  # head pairs per group = 2

F32 = mybir.dt.float32
F32R = mybir.dt.float32r
BF16 = mybir.dt.bfloat16
FP16 = mybir.dt.float16
DT_MM = BF16  # matmul operand dtype: BF16 (fast ldweights) or F32R (accuracy)
NP_MM = "bfloat16"  # host-side dtype name for DT_MM inputs
Exp = mybir.ActivationFunctionType.Exp
MUL = mybir.AluOpType.mult
ADD = mybir.AluOpType.add

NKVT = SKV // 128  # 32 kv tiles
NKVB = 8  # kv blocks (512 wide)
NFT_Q = 4096 // 128  # 32 contraction tiles for Q proj
NFT_KV = HID // 128  # 8 contraction tiles for K/V proj
NSQH = SQ // 512  # 2 sq halves
NJT = DOUT // 128  # 32 output row tiles

_NC_CACHE = {}


def build():
    if "nc" in _NC_CACHE:
        return _NC_CACHE["nc"]
    nc = bass.Bass()

    fft = nc.declare_dram_parameter("fft", [4096, SQ], DT_MM, isOutput=False)
    kvt = nc.declare_dram_parameter("kvt", [HID, SKV], DT_MM, isOutput=False)
    wqt = nc.declare_dram_parameter("wqt", [4096, GF], DT_MM, isOutput=False)
    wkt = nc.declare_dram_parameter("wkt", [HID, GF], DT_MM, isOutput=False)
    wvt = nc.declare_dram_parameter("wvt", [HID, GF], DT_MM, isOutput=False)
    wot = nc.declare_dram_parameter("wot", [GF, DOUT], DT_MM, isOutput=False)
    bq = nc.declare_dram_parameter("bq", [128, NPAIR], F32, isOutput=False)
    bk = nc.declare_dram_parameter("bk", [128, NPAIR], F32, isOutput=False)
    bv = nc.declare_dram_parameter("bv", [128, NPAIR], F32, isOutput=False)
    outp = nc.declare_dram_parameter("outp", [DOUT, SQ], FP16, isOutput=True)

    with tile.TileContext(nc) as tc:
        with (
            tc.tile_pool(name="hold", bufs=1) as hold,
            tc.tile_pool(name="misc", bufs=1) as misc,
        ):
            # ---- long-lived tiles ----
            wkt_r = hold.tile([128, NFT_KV, NPAIR, 128], DT_MM, tag="wkt")
            nc.sync.dma_start(
                out=wkt_r[:],
                in_=wkt.rearrange("(ft p) (pr d) -> p ft pr d", p=128, pr=NPAIR),
            )
            wvt_r = hold.tile([128, NFT_KV, GF], DT_MM, tag="wvt")
            nc.sync.dma_start(
                out=wvt_r[:], in_=wvt.rearrange("(ft p) d -> p ft d", p=128)
            )
            wot_r = hold.tile([128, NPAIR, DOUT], DT_MM, tag="wot")
            nc.sync.dma_start(
                out=wot_r[:], in_=wot.rearrange("(pr p) j -> p pr j", p=128)
            )
            bq_t = misc.tile([128, NPAIR], F32, tag="bq")
            nc.sync.dma_start(out=bq_t[:], in_=bq[:])
            bk_t = misc.tile([128, NPAIR], F32, tag="bk")
            nc.sync.dma_start(out=bk_t[:], in_=bk[:])
            bv_t = misc.tile([128, NPAIR], F32, tag="bv")
            nc.sync.dma_start(out=bv_t[:], in_=bv[:])

            ones_f = misc.tile([128, GHEADS], F32, tag="ones_f")
            nc.vector.memset(ones_f[:], 1.0)
            ones_row = misc.tile([1, DH], DT_MM, tag="ones_row")
            nc.vector.tensor_copy(ones_row[:], ones_f[0:1, 0:1].broadcast_to([1, DH]))

            qt_r = hold.tile([128, NPAIR, SQ], DT_MM, tag="qt")
            kt_r = hold.tile([128, NPAIR, SKV], DT_MM, tag="kt")
            v_r = hold.tile([128, NKVT, GHEADS, DH + 1], DT_MM, tag="v")
            att_r = hold.tile([128, NPAIR, SQ], DT_MM, tag="att")

            # ================= Phase A: QT projection =================
            with (
                nc.named_scope("phaseA_qt"),
                tc.tile_pool(name="ffts", bufs=6) as ffts,
                tc.tile_pool(name="wqs", bufs=4) as wqs,
                tc.tile_pool(name="psA", bufs=4, space="PSUM") as psA,
            ):
                qt_ps = [
                    psA.tile([128, 512], F32, tag="psA", name=f"qt_ps{i}")
                    for i in range(4)
                ]  # (pair, sqh)
                for kt in range(NFT_Q):
                    fft_t = ffts.tile([128, SQ], DT_MM, tag="fft")
                    nc.sync.dma_start(
                        out=fft_t[:], in_=fft[128 * kt : 128 * (kt + 1), :]
                    )
                    wq_t = wqs.tile([128, NPAIR, 128], DT_MM, tag="wq")
                    nc.sync.dma_start(
                        out=wq_t[:],
                        in_=wqt[128 * kt : 128 * (kt + 1), :].rearrange(
                            "p (pr d) -> p pr d", pr=NPAIR
                        ),
                    )
                    for pr in range(NPAIR):
                        for sh in range(NSQH):
                            nc.tensor.matmul(
                                qt_ps[pr * NSQH + sh][:],
                                wq_t[:, pr, :],
                                fft_t[:, 512 * sh : 512 * (sh + 1)],
                                start=(kt == 0),
                                stop=(kt == NFT_Q - 1),
                            )
                for pr in range(NPAIR):
                    for sh in range(NSQH):
                        nc.vector.tensor_scalar(
                            qt_r[:, pr, 512 * sh : 512 * (sh + 1)],
                            qt_ps[pr * NSQH + sh][:],
                            bq_t[:, pr : pr + 1],
                            None,
                            ADD,
                        )

            # ============ Phase B: KT + V projections (kv blocks) ============
            with (
                nc.named_scope("phaseB_kv"),
                tc.tile_pool(name="kvs", bufs=2) as kvs,
                tc.tile_pool(name="psB", bufs=4, space="PSUM") as psB,
            ):
                for kb in range(NKVB):
                    kv_t = kvs.tile([128, NFT_KV, 512], DT_MM, tag="kv")
                    nc.sync.dma_start(
                        out=kv_t[:],
                        in_=kvt[:, 512 * kb : 512 * (kb + 1)].rearrange(
                            "(ft p) n -> p ft n", p=128
                        ),
                    )
                    for pr in range(NPAIR):
                        kt_ps = psB.tile([128, 512], F32, tag="psB")
                        for ft in range(NFT_KV):
                            nc.tensor.matmul(
                                kt_ps[:],
                                wkt_r[:, ft, pr, :],
                                kv_t[:, ft, :],
                                start=(ft == 0),
                                stop=(ft == NFT_KV - 1),
                            )
                        nc.vector.tensor_scalar(
                            kt_r[:, pr, 512 * kb : 512 * (kb + 1)],
                            kt_ps[:],
                            bk_t[:, pr : pr + 1],
                            None,
                            ADD,
                        )
                    for kl in range(4):
                        kvt_i = kb * 4 + kl
                        v_ps = psB.tile([128, GF], F32, tag="psB")
                        for ft in range(NFT_KV):
                            nc.tensor.matmul(
                                v_ps[:],
                                kv_t[:, ft, 128 * kl : 128 * (kl + 1)],
                                wvt_r[:, ft, :],
                                start=(ft == 0),
                                stop=(ft == NFT_KV - 1),
                            )
                        nc.vector.tensor_copy(
                            v_r[:, kvt_i, :, 0:DH],
                            v_ps.rearrange("p (h d) -> p h d", h=GHEADS),
                        )
                        nc.vector.tensor_copy(
                            v_r[:, kvt_i, :, DH : DH + 1], ones_f[:, :]
                        )

            # ================= Phase C: attention =================
            # Per (pair, head): kv-loop with both sq halves sharing one
            # LDWEIGHTS and one [128, 2x512] score psum tile (2 banks,
            # double buffered) so each exp covers 1024 elements/lane.
            # PV runs one kv-tile behind scores (software pipeline).
            with (
                nc.named_scope("phaseC_attn"),
                tc.tile_pool(name="pp", bufs=3) as pp,
                tc.tile_pool(name="nrm", bufs=2) as nrm,
                tc.tile_pool(name="psS", bufs=2, space="PSUM") as psS,
                tc.tile_pool(name="psAtt", bufs=2, space="PSUM") as psAtt,
            ):
                for pr in range(NPAIR):
                    for h in range(2):
                        hd = 2 * pr + h
                        d_sl = slice(DH * h, DH * (h + 1))
                        att_ps = [
                            psAtt.tile(
                                [DH + 1, 512], F32, tag="att",
                                name=f"att{hd}_{sh}",
                            )
                            for sh in range(NSQH)
                        ]

                        def pv(kv, p, att_ps=att_ps, hd=hd):
                            for sh in range(NSQH):
                                nc.tensor.matmul(
                                    att_ps[sh][:],
                                    v_r[:, kv, hd, :],
                                    p[:, sh, :],
                                    start=(kv == 0),
                                    stop=(kv == NKVT - 1),
                                )

                        pq = []  # pending (kv, p) awaiting PV
                        for kv in range(NKVT):
                            s_ps = psS.tile(
                                [128, NSQH, 512], F32, tag="s", name=f"s{hd}_{kv}"
                            )
                            kv_sl = slice(128 * kv, 128 * (kv + 1))
                            for sh in range(NSQH):
                                nc.tensor.matmul(
                                    s_ps[:, sh, :],
                                    kt_r[d_sl, pr, kv_sl],
                                    qt_r[d_sl, pr, 512 * sh : 512 * (sh + 1)],
                                    start=True,
                                    stop=True,
                                )
                            p = pp.tile(
                                [128, NSQH, 512], DT_MM, tag="p", name=f"p{hd}_{kv}"
                            )
                            nc.scalar.activation(p[:], s_ps[:], Exp, scale=0.125)
                            pq.append((kv, p))
                            if kv >= 1:
                                pv(*pq.pop(0))
                        pv(*pq.pop(0))

                        # normalize per sh
                        for sh in range(NSQH):
                            att_h = att_ps[sh]
                            sq_sl = slice(512 * sh, 512 * (sh + 1))
                            rec = nrm.tile(
                                [1, 512], DT_MM, tag="rec", name=f"rec{hd}{sh}"
                            )
                            with nc.allow_low_precision(reason="softmax recip"):
                                nc.vector.reciprocal(rec[:], att_h[DH : DH + 1, :])
                            rb = psS.tile(
                                [DH, 512], F32, tag="s", name=f"rb{hd}{sh}"
                            )
                            nc.tensor.matmul(
                                rb[:], ones_row[0:1, :], rec[0:1, :],
                                start=True, stop=True,
                            )
                            rb_sb = nrm.tile(
                                [DH, 512], F32, tag="rbsb", name=f"rbsb{hd}{sh}"
                            )
                            nc.vector.tensor_copy(rb_sb[:], rb[:])
                            mulx = nrm.tile(
                                [DH, 512], F32, tag="mulx", name=f"mulx{hd}{sh}"
                            )
                            nc.vector.tensor_tensor(
                                mulx[:], att_h[0:DH, :], rb_sb[:], MUL
                            )
                            nc.vector.tensor_scalar(
                                att_r[64 * h : 64 * (h + 1), pr, sq_sl],
                                mulx[:],
                                bv_t[64 * h : 64 * (h + 1), pr : pr + 1],
                                None,
                                ADD,
                            )

            # ================= Phase D: out projection =================
            with (
                nc.named_scope("phaseD_out"),
                tc.tile_pool(name="osb", bufs=3) as osb,
                tc.tile_pool(name="psD", bufs=4, space="PSUM") as psD,
            ):
                for jt in range(NJT):
                    o_ps = [psD.tile([128, 512], F32, tag="psD", name=f"o_ps{jt}_{i}") for i in range(NSQH)]
                    j_sl = slice(128 * jt, 128 * (jt + 1))
                    for pr in range(NPAIR):
                        for sh in range(NSQH):
                            nc.tensor.matmul(
                                o_ps[sh][:],
                                wot_r[:, pr, j_sl],
                                att_r[:, pr, 512 * sh : 512 * (sh + 1)],
                                start=(pr == 0),
                                stop=(pr == NPAIR - 1),
                            )
                    o_sb = osb.tile([128, SQ], FP16, tag="osb")
                    for sh in range(NSQH):
                        nc.vector.tensor_copy(
                            o_sb[:, 512 * sh : 512 * (sh + 1)], o_ps[sh][:]
                        )
                    nc.sync.dma_start(out=outp[j_sl, :], in_=o_sb[:])

    _NC_CACHE["nc"] = nc
    return nc


def _make_in_maps(inputs):
    ff = np.asarray(inputs["fused_features"], dtype=np.float32)
    kv_in = np.concatenate(
        [
            np.asarray(inputs["text"], dtype=np.float32),
            np.asarray(inputs["image"], dtype=np.float32),
            np.asarray(inputs["audio"], dtype=np.float32),
            np.asarray(inputs["video"], dtype=np.float32),
        ],
        axis=1,
    )
    Wq = np.asarray(inputs["Wq"], dtype=np.float32)
    Wk = np.asarray(inputs["Wk"], dtype=np.float32)
    Wv = np.asarray(inputs["Wv"], dtype=np.float32)
    Wo = np.asarray(inputs["Wo"], dtype=np.float32)
    bq = np.asarray(inputs["bq"], dtype=np.float32)
    bk = np.asarray(inputs["bk"], dtype=np.float32)
    bv = np.asarray(inputs["bv"], dtype=np.float32)

    import ml_dtypes

    np_mm = np.dtype(ml_dtypes.bfloat16) if NP_MM == "bfloat16" else np.float32
    ffT = [np.ascontiguousarray(ff[b].T.astype(np_mm)) for b in range(B)]
    kvT = [np.ascontiguousarray(kv_in[b].T.astype(np_mm)) for b in range(B)]
    WqT = np.ascontiguousarray(Wq.T.astype(np_mm))  # [4096, 1024]
    WkT = np.ascontiguousarray(Wk.T.astype(np_mm))  # [1024, 1024]
    WvT = np.ascontiguousarray(Wv.T.astype(np_mm))
    WoT = np.ascontiguousarray(Wo.T.astype(np_mm))  # [1024, 4096]

    in_maps = []
    for c in range(NCORES):
        b, hg = divmod(c, HG)
        fs = slice(GF * hg, GF * (hg + 1))
        in_maps.append(
            {
                "fft": ffT[b],
                "kvt": kvT[b],
                "wqt": np.ascontiguousarray(WqT[:, fs]),
                "wkt": np.ascontiguousarray(WkT[:, fs]),
                "wvt": np.ascontiguousarray(WvT[:, fs]),
                "wot": np.ascontiguousarray(WoT[fs, :]),
                "bq": np.ascontiguousarray(bq[fs].reshape(NPAIR, 128).T),
                "bk": np.ascontiguousarray(bk[fs].reshape(NPAIR, 128).T),
                "bv": np.ascontiguousarray(bv[fs].reshape(NPAIR, 128).T),
            }
        )
    return in_maps


def _assemble(results, bo):
    out = np.zeros((B, SQ, DOUT), dtype=np.float32)
    for c in range(NCORES):
        b = c // HG
        out[b] += results[c]["outp"].T.astype(np.float32)
    out += np.asarray(bo, dtype=np.float32)
    return out


def run_spmd(inputs, trace=False):
    nc = build()
    in_maps = _make_in_maps(inputs)
    r = run_bass_kernel_spmd(nc, in_maps, list(range(NCORES)), trace=trace)
    return _assemble(r.results, inputs["bo"]), r


def kernel(**inputs) -> np.ndarray:
    out, _ = run_spmd(inputs, trace=False)
    return out


# revision 11
# speedup vs baseline: 1.2324x; 1.2319x over previous
"""Multi-head cross-attention TRN2 Bass kernel, sharded over 8 NeuronCores.

Problem (nn_MultiHeadCrossAttention): B=2, Sq=1024, Skv=4096 (text+image+
audio+video), hidden=1024, heads=16, head_dim=64, out=4096.

Sharding: core c = 4*b + hg handles batch b and head-group hg (4 heads).
Per core (all matmuls in float32r: ~bf16 speed, ~1e-4 accuracy):
  QT proj:  QT[d,sq]   = Wq_g  @ ff[b].T      (ffT streamed, contraction 4096)
  KT proj:  KT[d,kv]   = Wk_g  @ kv[b].T      (kvT streamed, contraction 1024)
  V  proj:  V[kv,d]    = kv[b] @ Wv_g.T       (natural layout, 65th col = ones)
  scores^T: S[kv,sq]   = K^T q  (row-tiled K=64 matmul pairs)
  softmax:  P = exp(S/8) (no max-subtract: |scores| <~ 3 for this data)
  PV:       att[d,sq] += V_ext^T @ P  (M=65: row 64 accumulates denominator)
  norm:     att = att * recip(den) (K=1 broadcast matmul expands recip row)
  out-proj: outT[j,sq] = Wo[:, fslice].T.T @ attT  -> partial over f-slice
Host sums the 4 per-batch partials and adds bo.
"""

import numpy as np

import bass_rust
import concourse.bass as bass
import concourse.mybir as mybir
import concourse.tile as tile
from concourse.bass_utils import run_bass_kernel_spmd
from concourse.vector_clock import ScopedClock

# ---------------------------------------------------------------------------
# Workarounds for walrus per-instruction sync-wait caps (this walrus build
# rejects instructions carrying more waits than the ISA slot count; Tile's
# sem assignment can attach more). Split excess waits onto single-wait nops.
# ---------------------------------------------------------------------------
import re as _re

_VC_RE = _re.compile(r"VectorClock\(\[([0-9, ]*)\]\)")


def _vc_values(vc):
    m = _VC_RE.match(repr(vc))
    assert m, repr(vc)
    s = m.group(1).strip()
    return [int(x) for x in s.split(",")] if s else []


def _split_excess_waits(tc, ordered_instructions_by_block, max_waits=1):
    nc = tc.nc
    for _bb, insts in ordered_instructions_by_block.items():
        out = []
        for inst in insts:
            si = inst.sync_info
            waits = list(si.on_wait) if si and si.on_wait else []
            if len(waits) > max_waits:
                keep = waits[:max_waits]
                for w in waits[max_waits:]:
                    nop = mybir.InstNoOp(
                        name=nc.get_next_instruction_name(), ins=[], outs=[]
                    )
                    nop.engine = inst.engine
                    nop.sync_info = bass_rust.SyncInfo(on_wait=[w], on_update=[])
                    nc.register_instruction(nop)
                    out.append(nop)
                inst.sync_info = bass_rust.SyncInfo(
                    on_wait=keep, on_update=list(si.on_update or [])
                )
            out.append(inst)
        insts[:] = out


_orig_lower = tile.TileContext._lower_ordered_insts


def _lower_with_split(self, postordered_blocks):
    _split_excess_waits(self, postordered_blocks)
    return _orig_lower(self, postordered_blocks)


def _drain_and_barrier_split(self, tick_clock, wait_clock):
    vals = _vc_values(tick_clock.global_clock)
    for proc_idx, tick in enumerate(vals):
        if tick <= 0:
            continue
        single = [0] * len(vals)
        single[proc_idx] = tick
        nop_inst = self.nc.sync.nop(nofuse=True, hint=f"drain_wait_p{proc_idx}")
        wait_clock.add_sem_waits(
            nop_inst.ins, ScopedClock({None: bass_rust.VectorClock(single)})
        )
    self.nc.sync.drain()
    self.nc.all_engine_barrier()
    assert self.sems is not None
    popped = self.nc._tile_sem_poison_stack.pop()
    assert popped is self._sem_poison
    self.nc.clear_and_free_semaphores(list(self.sems.allocated().values()))
    self.nc.all_engine_barrier()


tile.TileContext._lower_ordered_insts = _lower_with_split
tile.TileContext._drain_and_barrier = _drain_and_barrier_split

# ---------------------------------------------------------------------------
# Problem constants (hardcoded per contract)
# ---------------------------------------------------------------------------
B = 2
SQ = 1024
SKV = 4096
HID = 1024
HEADS = 16
DH = 64
DOUT = 4096
NCORES = 8
HG = 4  # head-groups (cores per batch)
GHEADS = HEADS // HG  # heads per group = 4
GF = GHEADS * DH  # feature slice per group = 256
NPAIR = GHEADS // 2  # head pairs per group = 2

F32 = mybir.dt.float32
F32R = mybir.dt.float32r
BF16 = mybir.dt.bfloat16
FP16 = mybir.dt.float16
DT_MM = BF16  # matmul operand dtype: BF16 (fast ldweights) or F32R (accuracy)
NP_MM = "bfloat16"  # host-side dtype name for DT_MM inputs
Exp = mybir.ActivationFunctionType.Exp
MUL = mybir.AluOpType.mult
ADD = mybir.AluOpType.add

NKVT = SKV // 128  # 32 kv tiles
NKVB = 8  # kv blocks (512 wide)
NFT_Q = 4096 // 128  # 32 contraction tiles for Q proj
NFT_KV = HID // 128  # 8 contraction tiles for K/V proj
NSQH = SQ // 512  # 2 sq halves
NJT = DOUT // 128  # 32 output row tiles

_NC_CACHE = {}


def build():
    if "nc" in _NC_CACHE:
        return _NC_CACHE["nc"]
    nc = bass.Bass()

    fft = nc.declare_dram_parameter("fft", [4096, SQ], DT_MM, isOutput=False)
    kvt = nc.declare_dram_parameter("kvt", [HID, SKV], DT_MM, isOutput=False)
    wqt = nc.declare_dram_parameter("wqt", [4096, GF], DT_MM, isOutput=False)
    wkt = nc.declare_dram_parameter("wkt", [HID, GF], DT_MM, isOutput=False)
    wvt = nc.declare_dram_parameter("wvt", [HID, GF], DT_MM, isOutput=False)
    wot = nc.declare_dram_parameter("wot", [GF, DOUT], DT_MM, isOutput=False)
    bq = nc.declare_dram_parameter("bq", [128, NPAIR], F32, isOutput=False)
    bk = nc.declare_dram_parameter("bk", [128, NPAIR], F32, isOutput=False)
    bv = nc.declare_dram_parameter("bv", [128, NPAIR], F32, isOutput=False)
    outp = nc.declare_dram_parameter("outp", [DOUT, SQ], FP16, isOutput=True)

    with tile.TileContext(nc) as tc:
        with (
            tc.tile_pool(name="hold", bufs=1) as hold,
            tc.tile_pool(name="misc", bufs=1) as misc,
        ):
            # ---- long-lived tiles ----
            wkt_r = hold.tile([128, NFT_KV, NPAIR, 128], DT_MM, tag="wkt")
            nc.sync.dma_start(
                out=wkt_r[:],
                in_=wkt.rearrange("(ft p) (pr d) -> p ft pr d", p=128, pr=NPAIR),
            )
            wvt_r = hold.tile([128, NFT_KV, GF], DT_MM, tag="wvt")
            nc.sync.dma_start(
                out=wvt_r[:], in_=wvt.rearrange("(ft p) d -> p ft d", p=128)
            )
            wot_r = hold.tile([128, NPAIR, DOUT], DT_MM, tag="wot")
            nc.sync.dma_start(
                out=wot_r[:], in_=wot.rearrange("(pr p) j -> p pr j", p=128)
            )
            bq_t = misc.tile([128, NPAIR], F32, tag="bq")
            nc.sync.dma_start(out=bq_t[:], in_=bq[:])
            bk_t = misc.tile([128, NPAIR], F32, tag="bk")
            nc.sync.dma_start(out=bk_t[:], in_=bk[:])
            bv_t = misc.tile([128, NPAIR], F32, tag="bv")
            nc.sync.dma_start(out=bv_t[:], in_=bv[:])

            ones_f = misc.tile([128, GHEADS], F32, tag="ones_f")
            nc.vector.memset(ones_f[:], 1.0)
            ones_row = misc.tile([1, DH], DT_MM, tag="ones_row")
            nc.vector.tensor_copy(ones_row[:], ones_f[0:1, 0:1].broadcast_to([1, DH]))

            qt_r = hold.tile([128, GHEADS, SQ], DT_MM, tag="qt")
            kt_r = hold.tile([128, GHEADS, SKV], DT_MM, tag="kt")
            v_r = hold.tile([128, NKVT, GHEADS, DH + 1], DT_MM, tag="v")
            att_r = hold.tile([128, NPAIR, SQ], DT_MM, tag="att")

            # ================= Phase A: QT projection =================
            with (
                nc.named_scope("phaseA_qt"),
                tc.tile_pool(name="ffts", bufs=6) as ffts,
                tc.tile_pool(name="wqs", bufs=4) as wqs,
                tc.tile_pool(name="psA", bufs=4, space="PSUM") as psA,
            ):
                qt_ps = [
                    psA.tile([128, 512], F32, tag="psA", name=f"qt_ps{i}")
                    for i in range(4)
                ]  # (pair, sqh)
                for kt in range(NFT_Q):
                    fft_t = ffts.tile([128, SQ], DT_MM, tag="fft")
                    nc.sync.dma_start(
                        out=fft_t[:], in_=fft[128 * kt : 128 * (kt + 1), :]
                    )
                    wq_t = wqs.tile([128, NPAIR, 128], DT_MM, tag="wq")
                    nc.sync.dma_start(
                        out=wq_t[:],
                        in_=wqt[128 * kt : 128 * (kt + 1), :].rearrange(
                            "p (pr d) -> p pr d", pr=NPAIR
                        ),
                    )
                    for pr in range(NPAIR):
                        for sh in range(NSQH):
                            nc.tensor.matmul(
                                qt_ps[pr * NSQH + sh][:],
                                wq_t[:, pr, :],
                                fft_t[:, 512 * sh : 512 * (sh + 1)],
                                start=(kt == 0),
                                stop=(kt == NFT_Q - 1),
                            )
                for pr in range(NPAIR):
                    for sh in range(NSQH):
                        sq_sl = slice(512 * sh, 512 * (sh + 1))
                        nc.vector.tensor_scalar(
                            qt_r[0:64, 2 * pr, sq_sl],
                            qt_ps[pr * NSQH + sh][0:64, :],
                            bq_t[0:64, pr : pr + 1],
                            None,
                            ADD,
                        )
                        nc.vector.tensor_scalar(
                            qt_r[64:128, 2 * pr + 1, sq_sl],
                            qt_ps[pr * NSQH + sh][64:128, :],
                            bq_t[64:128, pr : pr + 1],
                            None,
                            ADD,
                        )
                # duplicate halves so score matmuls contract K=128 (2x scores,
                # folded into the exp scale) -- keeps the PE array fully
                # row-active so HAM stays at full clock.
                for pr in range(NPAIR):
                    nc.sync.dma_start(
                        out=qt_r[64:128, 2 * pr, :], in_=qt_r[0:64, 2 * pr, :]
                    )
                    nc.sync.dma_start(
                        out=qt_r[0:64, 2 * pr + 1, :],
                        in_=qt_r[64:128, 2 * pr + 1, :],
                    )

            # ============ Phase B: KT + V projections (kv blocks) ============
            with (
                nc.named_scope("phaseB_kv"),
                tc.tile_pool(name="kvs", bufs=2) as kvs,
                tc.tile_pool(name="psB", bufs=4, space="PSUM") as psB,
            ):
                for kb in range(NKVB):
                    kv_t = kvs.tile([128, NFT_KV, 512], DT_MM, tag="kv")
                    nc.sync.dma_start(
                        out=kv_t[:],
                        in_=kvt[:, 512 * kb : 512 * (kb + 1)].rearrange(
                            "(ft p) n -> p ft n", p=128
                        ),
                    )
                    for pr in range(NPAIR):
                        kt_ps = psB.tile([128, 512], F32, tag="psB")
                        for ft in range(NFT_KV):
                            nc.tensor.matmul(
                                kt_ps[:],
                                wkt_r[:, ft, pr, :],
                                kv_t[:, ft, :],
                                start=(ft == 0),
                                stop=(ft == NFT_KV - 1),
                            )
                        kb_sl = slice(512 * kb, 512 * (kb + 1))
                        nc.vector.tensor_scalar(
                            kt_r[0:64, 2 * pr, kb_sl],
                            kt_ps[0:64, :],
                            bk_t[0:64, pr : pr + 1],
                            None,
                            ADD,
                        )
                        nc.vector.tensor_scalar(
                            kt_r[64:128, 2 * pr + 1, kb_sl],
                            kt_ps[64:128, :],
                            bk_t[64:128, pr : pr + 1],
                            None,
                            ADD,
                        )
                        nc.sync.dma_start(
                            out=kt_r[64:128, 2 * pr, kb_sl],
                            in_=kt_r[0:64, 2 * pr, kb_sl],
                        )
                        nc.sync.dma_start(
                            out=kt_r[0:64, 2 * pr + 1, kb_sl],
                            in_=kt_r[64:128, 2 * pr + 1, kb_sl],
                        )
                    for kl in range(4):
                        kvt_i = kb * 4 + kl
                        v_ps = psB.tile([128, GF], F32, tag="psB")
                        for ft in range(NFT_KV):
                            nc.tensor.matmul(
                                v_ps[:],
                                kv_t[:, ft, 128 * kl : 128 * (kl + 1)],
                                wvt_r[:, ft, :],
                                start=(ft == 0),
                                stop=(ft == NFT_KV - 1),
                            )
                        nc.vector.tensor_copy(
                            v_r[:, kvt_i, :, 0:DH],
                            v_ps.rearrange("p (h d) -> p h d", h=GHEADS),
                        )
                        nc.vector.tensor_copy(
                            v_r[:, kvt_i, :, DH : DH + 1], ones_f[:, :]
                        )

            # ================= Phase C: attention =================
            # Per (pair, head): kv-loop with both sq halves sharing one
            # LDWEIGHTS and one [128, 2x512] score psum tile (2 banks,
            # double buffered) so each exp covers 1024 elements/lane.
            # PV runs one kv-tile behind scores (software pipeline).
            with (
                nc.named_scope("phaseC_attn"),
                tc.tile_pool(name="pp", bufs=3) as pp,
                tc.tile_pool(name="nrm", bufs=2) as nrm,
                tc.tile_pool(name="psS", bufs=2, space="PSUM") as psS,
                tc.tile_pool(name="psAtt", bufs=2, space="PSUM") as psAtt,
            ):
                for pr in range(NPAIR):
                    for h in range(2):
                        hd = 2 * pr + h
                        att_ps = [
                            psAtt.tile(
                                [DH + 1, 512], F32, tag="att",
                                name=f"att{hd}_{sh}",
                            )
                            for sh in range(NSQH)
                        ]

                        def pv(kv, p, att_ps=att_ps, hd=hd):
                            for sh in range(NSQH):
                                nc.tensor.matmul(
                                    att_ps[sh][:],
                                    v_r[:, kv, hd, :],
                                    p[:, sh, :],
                                    start=(kv == 0),
                                    stop=(kv == NKVT - 1),
                                )

                        pq = []  # pending (kv, p) awaiting PV
                        for kv in range(NKVT):
                            s_ps = psS.tile(
                                [128, NSQH, 512], F32, tag="s", name=f"s{hd}_{kv}"
                            )
                            kv_sl = slice(128 * kv, 128 * (kv + 1))
                            for sh in range(NSQH):
                                nc.tensor.matmul(
                                    s_ps[:, sh, :],
                                    kt_r[:, hd, kv_sl],
                                    qt_r[:, hd, 512 * sh : 512 * (sh + 1)],
                                    start=True,
                                    stop=True,
                                )
                            p = pp.tile(
                                [128, NSQH, 512], DT_MM, tag="p", name=f"p{hd}_{kv}"
                            )
                            nc.scalar.activation(p[:], s_ps[:], Exp, scale=0.0625)
                            pq.append((kv, p))
                            if kv >= 1:
                                pv(*pq.pop(0))
                        pv(*pq.pop(0))

                        # normalize per sh
                        for sh in range(NSQH):
                            att_h = att_ps[sh]
                            sq_sl = slice(512 * sh, 512 * (sh + 1))
                            rec = nrm.tile(
                                [1, 512], DT_MM, tag="rec", name=f"rec{hd}{sh}"
                            )
                            with nc.allow_low_precision(reason="softmax recip"):
                                nc.vector.reciprocal(rec[:], att_h[DH : DH + 1, :])
                            rb = psS.tile(
                                [DH, 512], F32, tag="s", name=f"rb{hd}{sh}"
                            )
                            nc.tensor.matmul(
                                rb[:], ones_row[0:1, :], rec[0:1, :],
                                start=True, stop=True,
                            )
                            rb_sb = nrm.tile(
                                [DH, 512], F32, tag="rbsb", name=f"rbsb{hd}{sh}"
                            )
                            nc.vector.tensor_copy(rb_sb[:], rb[:])
                            mulx = nrm.tile(
                                [DH, 512], F32, tag="mulx", name=f"mulx{hd}{sh}"
                            )
                            nc.vector.tensor_tensor(
                                mulx[:], att_h[0:DH, :], rb_sb[:], MUL
                            )
                            nc.vector.tensor_scalar(
                                att_r[64 * h : 64 * (h + 1), pr, sq_sl],
                                mulx[:],
                                bv_t[64 * h : 64 * (h + 1), pr : pr + 1],
                                None,
                                ADD,
                            )

            # ================= Phase D: out projection =================
            with (
                nc.named_scope("phaseD_out"),
                tc.tile_pool(name="osb", bufs=3) as osb,
                tc.tile_pool(name="psD", bufs=4, space="PSUM") as psD,
            ):
                for jt in range(NJT):
                    o_ps = [psD.tile([128, 512], F32, tag="psD", name=f"o_ps{jt}_{i}") for i in range(NSQH)]
                    j_sl = slice(128 * jt, 128 * (jt + 1))
                    for pr in range(NPAIR):
                        for sh in range(NSQH):
                            nc.tensor.matmul(
                                o_ps[sh][:],
                                wot_r[:, pr, j_sl],
                                att_r[:, pr, 512 * sh : 512 * (sh + 1)],
                                start=(pr == 0),
                                stop=(pr == NPAIR - 1),
                            )
                    o_sb = osb.tile([128, SQ], FP16, tag="osb")
                    for sh in range(NSQH):
                        nc.vector.tensor_copy(
                            o_sb[:, 512 * sh : 512 * (sh + 1)], o_ps[sh][:]
                        )
                    nc.sync.dma_start(out=outp[j_sl, :], in_=o_sb[:])

    _NC_CACHE["nc"] = nc
    return nc


def _make_in_maps(inputs):
    ff = np.asarray(inputs["fused_features"], dtype=np.float32)
    kv_in = np.concatenate(
        [
            np.asarray(inputs["text"], dtype=np.float32),
            np.asarray(inputs["image"], dtype=np.float32),
            np.asarray(inputs["audio"], dtype=np.float32),
            np.asarray(inputs["video"], dtype=np.float32),
        ],
        axis=1,
    )
    Wq = np.asarray(inputs["Wq"], dtype=np.float32)
    Wk = np.asarray(inputs["Wk"], dtype=np.float32)
    Wv = np.asarray(inputs["Wv"], dtype=np.float32)
    Wo = np.asarray(inputs["Wo"], dtype=np.float32)
    bq = np.asarray(inputs["bq"], dtype=np.float32)
    bk = np.asarray(inputs["bk"], dtype=np.float32)
    bv = np.asarray(inputs["bv"], dtype=np.float32)

    import ml_dtypes

    np_mm = np.dtype(ml_dtypes.bfloat16) if NP_MM == "bfloat16" else np.float32
    ffT = [np.ascontiguousarray(ff[b].T.astype(np_mm)) for b in range(B)]
    kvT = [np.ascontiguousarray(kv_in[b].T.astype(np_mm)) for b in range(B)]
    WqT = np.ascontiguousarray(Wq.T.astype(np_mm))  # [4096, 1024]
    WkT = np.ascontiguousarray(Wk.T.astype(np_mm))  # [1024, 1024]
    WvT = np.ascontiguousarray(Wv.T.astype(np_mm))
    WoT = np.ascontiguousarray(Wo.T.astype(np_mm))  # [1024, 4096]

    in_maps = []
    for c in range(NCORES):
        b, hg = divmod(c, HG)
        fs = slice(GF * hg, GF * (hg + 1))
        in_maps.append(
            {
                "fft": ffT[b],
                "kvt": kvT[b],
                "wqt": np.ascontiguousarray(WqT[:, fs]),
                "wkt": np.ascontiguousarray(WkT[:, fs]),
                "wvt": np.ascontiguousarray(WvT[:, fs]),
                "wot": np.ascontiguousarray(WoT[fs, :]),
                "bq": np.ascontiguousarray(bq[fs].reshape(NPAIR, 128).T),
                "bk": np.ascontiguousarray(bk[fs].reshape(NPAIR, 128).T),
                "bv": np.ascontiguousarray(bv[fs].reshape(NPAIR, 128).T),
            }
        )
    return in_maps


def _assemble(results, bo):
    out = np.zeros((B, SQ, DOUT), dtype=np.float32)
    for c in range(NCORES):
        b = c // HG
        out[b] += results[c]["outp"].T.astype(np.float32)
    out += np.asarray(bo, dtype=np.float32)
    return out


def run_spmd(inputs, trace=False):
    nc = build()
    in_maps = _make_in_maps(inputs)
    r = run_bass_kernel_spmd(nc, in_maps, list(range(NCORES)), trace=trace)
    return _assemble(r.results, inputs["bo"]), r


def kernel(**inputs) -> np.ndarray:
    out, _ = run_spmd(inputs, trace=False)
    return out
